# revision 8
# baseline (speedup 1.0000x reference)
"""Trainium2 Bass kernel for nn_DynamicMaxSimilarity — v4.

Full inputs a,b: [512, 16, 256] f32.
  an = l2norm(tanh(a)) rows; bn likewise
  sim[a,b,i,j] = dot(an[a,i], bn[b,j]);  out[a,b] = DTW-like max-avg DP.

Sharding: 8 cores as 4 a-chunks (128) x 2 b-chunks (256). Per-core block
[128 a, 256 b]; pairs live as [128 partitions (a), 256 free (b)].

v4 vs baseline:
- fp16 normalize/matmul path (tanh casts to fp16; matmuls run fp16).
- XBAR DMA transposes (dma_start_transpose) replace PE transposes and
  their PSUM->SBUF evictions; DMA engines are otherwise idle.
- ACT table plan: all Tanh first, one switch to the Sqrt set, then only
  Copy/Square (in every set) -> 2 table loads instead of 14.
- sumsq via ACT Square+accum_out per frame (off the DVE critical path);
  row scales via fp16 tensor_scalar (4x DVE mode).
- DP unchanged math (scaled domain u = si*max(i,j), fp32 state) but
  single-slot chain injects: per pair chain slots [inject, data...]:
    inject: d0 = max(udp, m1*(k-1)/k) + 64 (ACT write, bias),
            d1 = lc_diag - 64 (diag eviction with bias=-64)
    scan:   y = max(d0, stale) + d1 = u[k,k]   (|u| <= 16 < 48 <= d0)
  which drops one slot per chain per step vs the baseline's two-slot
  scheme and removes the -BIG memsets and D2 copies.
"""

import numpy as np

import concourse.bass as bass
from concourse import bacc
import concourse.mybir as mybir
from concourse.tile import TileContext
from concourse import bass_utils

NA, NB, T, D = 512, 512, 16, 256
ACH, BCH = 128, 256
P = 128
F = BCH              # pairs per partition
KH = D // 128
DT = mybir.dt.float32
HT = mybir.dt.float16
ALU = mybir.AluOpType
ACTF = mybir.ActivationFunctionType
BIAS = 64.0

_last_results = None


def build_program():
    nc = bacc.Bacc("TRN2", target_bir_lowering=False, debug=False)

    a_d = nc.dram_tensor("a_c", [ACH, T, D], DT, kind="ExternalInput")
    b_d = nc.dram_tensor("b_c", [BCH, T, D], DT, kind="ExternalInput")
    out_d = nc.dram_tensor("out", [ACH, BCH], DT, kind="ExternalOutput")

    with TileContext(nc) as tc:
        with (
            tc.tile_pool(name="mp", bufs=1) as mp,
            tc.tile_pool(name="wp", bufs=2) as wp,
        ):
            # ---- load (fp32) ----
            a_sb = mp.tile([P, T, D], DT, tag="ld_a")
            for q in range(4):
                nc.sync.dma_start(a_sb[:, q * 4:(q + 1) * 4, :],
                                  a_d.ap()[:, q * 4:(q + 1) * 4, :])
            b_sb = [mp.tile([P, T, D], DT, name=f"b_sb{h}", tag=f"ld_b{h}")
                    for h in range(2)]
            for h in range(2):
                for q in range(4):
                    nc.sync.dma_start(
                        b_sb[h][:, q * 4:(q + 1) * 4, :],
                        b_d.ap()[h * 128:(h + 1) * 128, q * 4:(q + 1) * 4, :])

            # ---- tanh -> fp16; sumsq on ACT; rinv; fp16 scales ----
            ah = mp.tile([P, T, D], HT)
            bh = [mp.tile([P, T, D], HT, name=f"bh{h}") for h in range(2)]
            ssq = mp.tile([P, 3, T], DT)
            nrm = mp.tile([P, 3, T], DT)
            rinv = mp.tile([P, 3, T], DT)
            blocks = [(a_sb, ah, 0), (b_sb[0], bh[0], 1), (b_sb[1], bh[1], 2)]
            aT = mp.tile([P, T * KH, P], HT)       # [d, i*2+kh, a]
            bT = mp.tile([P, T, KH, 2, P], HT)     # [d, j, kh, half, b]
            # per-block pipeline: tanh (ACT) -> sumsq (DVE) -> sqrt (one ACT
            # instr per block) -> recip -> scales -> XBAR transposes, so each
            # block's transposed operands are ready as soon as its load is.
            for x_sb, xh, bi in blocks:
                for q in range(4):
                    sl = slice(q * 4, (q + 1) * 4)
                    nc.scalar.activation(xh[:, sl, :], x_sb[:, sl, :],
                                         ACTF.Tanh)
                    sq = wp.tile([P, 4, D], HT, name=f"sq{bi}_{q}", tag="sq")
                    nc.vector.tensor_tensor(sq[:, :, :], xh[:, sl, :],
                                            xh[:, sl, :], ALU.mult)
                    nc.vector.tensor_reduce(ssq[:, bi, sl], sq[:, :, :],
                                            mybir.AxisListType.X, ALU.add)
                nc.scalar.activation(nrm[:, bi, :], ssq[:, bi, :], ACTF.Sqrt)
                nc.vector.reciprocal(rinv[:, bi, :], nrm[:, bi, :])
                for q in range(4):
                    for i in range(q * 4, (q + 1) * 4):
                        nc.vector.tensor_scalar_mul(xh[:, i, :], xh[:, i, :],
                                                    rinv[:, bi, i:i + 1])
                    sl = slice(q * 4, (q + 1) * 4)
                    if bi == 0:
                        nc.sync.dma_start_transpose(
                            aT[:, q * 8:(q + 1) * 8, :], xh[:, sl, :])
                    else:
                        nc.sync.dma_start_transpose(
                            bT[:, sl, :, bi - 1, :], xh[:, sl, :])

            def amat(i0, kh):
                return aT[:, i0 * KH + kh, :]

            def bmov(j0, w, kh):
                return bT[:, j0:j0 + w, kh, :, :]

            # ---- DP state ----
            # Step k: S = 17-k slots per pair: [0 inject, c=1..16-k data
            # (cell j = k+c)]; regions packed [row | col], pair-major.
            RC0 = mp.tile([P, 2 * 16 * F], DT, tag="ld_a")
            RC1 = mp.tile([P, 2 * 16 * F], DT, tag="ld_b0")
            LRC = mp.tile([P, 2 * 16 * F], DT, tag="ld_b1")
            ud = mp.tile([P, F], DT)

            def reg(tile, k, w):
                S = 17 - k
                return tile[:, w * S * F:(w + 1) * S * F].rearrange(
                    "p (f s) -> p s f", s=S)

            def chain2d(tile, k, w):
                S = 17 - k
                return tile[:, w * S * F:(w + 1) * S * F]

            def both(tile, k):
                S = 17 - k
                return tile[:, 0:2 * S * F].rearrange(
                    "p (w f s) -> p w f s", w=2, s=S)

            rc_prev = None
            m1 = [wp.tile([P, F], DT, name=f"m1_{x}", tag="m1")
                  for x in range(2)]

            with tc.tile_pool(name="pp", bufs=1, space="PSUM") as pp:
                for k in range(1, T + 1):
                    S = 17 - k
                    nd = 16 - k          # data slots per region

                    # --- diag plane (a-frame k-1 x b-frame k-1) ---
                    psd = pp.tile([128, F], DT, name=f"psd{k}", tag="locd",
                                  bufs=2)
                    for kh in range(KH):
                        nc.tensor.matmul(psd[:, :], amat(k - 1, kh),
                                         bmov(k - 1, 1, kh),
                                         start=(kh == 0), stop=(kh == KH - 1))

                    rc_cur = RC0 if (k % 2) else RC1
                    if k == 1:
                        nc.gpsimd.memset(both(rc_cur, k)[:, :, :, 0:1], BIAS)
                        nc.gpsimd.memset(both(rc_cur, k)[:, :, :, 1:S], 0.0)
                    else:
                        mm = m1[k % 2]
                        nc.vector.tensor_tensor(
                            mm[:, :], reg(rc_prev, k - 1, 0)[:, 1, :],
                            reg(rc_prev, k - 1, 1)[:, 1, :], ALU.max)
                        nc.vector.scalar_tensor_tensor(
                            mm[:, :], mm[:, :], float((k - 1) / k),
                            reg(rc_prev, k - 1, 0)[:, 0, :],
                            ALU.mult, ALU.max)
                        if k == T:
                            # u[16,16] = max(m1*15/16, udp) + lc_diag
                            nc.vector.tensor_tensor(ud[:, :], mm[:, :],
                                                    psd[:, :], ALU.add)
                            break
                        # inject d0 = mm + 64 into slot 0 of both regions
                        # (on DVE: keeps ACT out of the per-step loop)
                        for w in range(2):
                            nc.vector.tensor_scalar_add(
                                reg(rc_cur, k, w)[:, 0, :], mm[:, :], BIAS)

                    # --- diag eviction: inject d1 = lc_diag - 64 ---
                    for w in range(2):
                        nc.scalar.activation(reg(LRC, k, w)[:, 0, :],
                                             psd[:, :], ACTF.Copy, bias=-BIAS)

                    # --- border matmuls + evictions ---
                    # row data planes: b-frames k-1+c (c=1..nd) -> slots c
                    for c0 in range(1, nd + 1, 2):
                        w = min(2, nd + 1 - c0)
                        psv = pp.tile([128, 2 * F], DT, name=f"psr{k}_{c0}",
                                      tag="locr", bufs=3)
                        for kh in range(KH):
                            nc.tensor.matmul(
                                psv[:, 0:w * F], amat(k - 1, kh),
                                bmov(k - 1 + c0, w, kh),
                                start=(kh == 0), stop=(kh == KH - 1))
                        nc.scalar.activation(
                            reg(LRC, k, 0)[:, c0:c0 + w, :],
                            psv[:, 0:w * F].rearrange("p (n f) -> p n f", f=F),
                            ACTF.Copy)
                    # col data planes: a-frames k-1+c (c=1..nd)
                    for c0 in range(1, nd + 1, 2):
                        w = min(2, nd + 1 - c0)
                        psv = pp.tile([128, 2 * F], DT, name=f"psc{k}_{c0}",
                                      tag="locc", bufs=3)
                        for q in range(w):
                            for kh in range(KH):
                                nc.tensor.matmul(
                                    psv[:, q * F:(q + 1) * F],
                                    amat(k - 1 + c0 + q, kh),
                                    bmov(k - 1, 1, kh),
                                    start=(kh == 0), stop=(kh == KH - 1))
                        nc.scalar.activation(
                            reg(LRC, k, 1)[:, c0:c0 + w, :],
                            psv[:, 0:w * F].rearrange("p (n f) -> p n f", f=F),
                            ACTF.Copy)

                    # --- preps: t[c] = max(prev[c+1]*(j-1)/j, prev[c]) ---
                    if k > 1:
                        bc = both(rc_cur, k)
                        bp = both(rc_prev, k - 1)
                        for c in range(1, nd + 1):
                            j = k + c
                            nc.vector.scalar_tensor_tensor(
                                bc[:, :, :, c], bp[:, :, :, c + 1],
                                float((j - 1) / j), bp[:, :, :, c],
                                ALU.mult, ALU.max)

                    # --- scan (both regions in one chain pass) ---
                    nc.vector.tensor_tensor_scan(
                        rc_cur[:, 0:2 * S * F], rc_cur[:, 0:2 * S * F],
                        LRC[:, 0:2 * S * F], 0.0, ALU.max, ALU.add)

                    rc_prev = rc_cur

            out_sb = mp.tile([P, F], DT)
            nc.vector.tensor_scalar_mul(out_sb[:, :], ud[:, :], 1.0 / T)
            nc.sync.dma_start(out_d.ap(), out_sb[:, :])

    nc.compile()
    return nc


def kernel(a: np.ndarray, b: np.ndarray) -> np.ndarray:
    a = np.ascontiguousarray(a, dtype=np.float32)
    b = np.ascontiguousarray(b, dtype=np.float32)
    assert a.shape == (NA, T, D) and b.shape == (NB, T, D)

    nc = build_program()

    in_maps = []
    for core in range(8):
        ca, cb = core // 2, core % 2
        in_maps.append({
            "a_c": a[ca * ACH:(ca + 1) * ACH],
            "b_c": b[cb * BCH:(cb + 1) * BCH],
        })

    res = bass_utils.run_bass_kernel_spmd(nc, in_maps, core_ids=list(range(8)))
    global _last_results
    _last_results = res

    out = np.zeros((NA, NB), dtype=np.float32)
    for core in range(8):
        ca, cb = core // 2, core % 2
        out[ca * ACH:(ca + 1) * ACH, cb * BCH:(cb + 1) * BCH] = \
            res.results[core]["out"]
    return out


# revision 11
# speedup vs baseline: 1.0223x; 1.0223x over previous
"""Trainium2 Bass kernel for nn_DynamicMaxSimilarity — v4.

Full inputs a,b: [512, 16, 256] f32.
  an = l2norm(tanh(a)) rows; bn likewise
  sim[a,b,i,j] = dot(an[a,i], bn[b,j]);  out[a,b] = DTW-like max-avg DP.

Sharding: 8 cores as 4 a-chunks (128) x 2 b-chunks (256). Per-core block
[128 a, 256 b]; pairs live as [128 partitions (a), 256 free (b)].

v4 vs baseline:
- fp16 normalize/matmul path (tanh casts to fp16; matmuls run fp16).
- XBAR DMA transposes (dma_start_transpose) replace PE transposes and
  their PSUM->SBUF evictions; DMA engines are otherwise idle.
- ACT table plan: all Tanh first, one switch to the Sqrt set, then only
  Copy/Square (in every set) -> 2 table loads instead of 14.
- sumsq via ACT Square+accum_out per frame (off the DVE critical path);
  row scales via fp16 tensor_scalar (4x DVE mode).
- DP unchanged math (scaled domain u = si*max(i,j), fp32 state) but
  single-slot chain injects: per pair chain slots [inject, data...]:
    inject: d0 = max(udp, m1*(k-1)/k) + 64 (ACT write, bias),
            d1 = lc_diag - 64 (diag eviction with bias=-64)
    scan:   y = max(d0, stale) + d1 = u[k,k]   (|u| <= 16 < 48 <= d0)
  which drops one slot per chain per step vs the baseline's two-slot
  scheme and removes the -BIG memsets and D2 copies.
"""

import numpy as np

import concourse.bass as bass
from concourse import bacc
import concourse.mybir as mybir
from concourse.tile import TileContext
from concourse import bass_utils

NA, NB, T, D = 512, 512, 16, 256
ACH, BCH = 128, 256
P = 128
F = BCH              # pairs per partition
KH = D // 128
DT = mybir.dt.float32
HT = mybir.dt.float16
ALU = mybir.AluOpType
ACTF = mybir.ActivationFunctionType
BIAS = 64.0

_last_results = None


def build_program():
    nc = bacc.Bacc("TRN2", target_bir_lowering=False, debug=False)

    a_d = nc.dram_tensor("a_c", [ACH, T, D], DT, kind="ExternalInput")
    b_d = nc.dram_tensor("b_c", [BCH, T, D], DT, kind="ExternalInput")
    out_d = nc.dram_tensor("out", [ACH, BCH], DT, kind="ExternalOutput")

    with TileContext(nc) as tc:
        with (
            tc.tile_pool(name="mp", bufs=1) as mp,
            tc.tile_pool(name="wp", bufs=2) as wp,
        ):
            # ---- load (fp32), quarters interleaved across blocks so every
            # block's normalize pipeline starts as early as possible ----
            a_sb = mp.tile([P, T, D], DT, tag="ld_a")
            b_sb = [mp.tile([P, T, D], DT, name=f"b_sb{h}", tag=f"ld_b{h}")
                    for h in range(2)]
            for q in range(4):
                nc.sync.dma_start(a_sb[:, q * 4:(q + 1) * 4, :],
                                  a_d.ap()[:, q * 4:(q + 1) * 4, :])
                for h in range(2):
                    nc.sync.dma_start(
                        b_sb[h][:, q * 4:(q + 1) * 4, :],
                        b_d.ap()[h * 128:(h + 1) * 128, q * 4:(q + 1) * 4, :])

            # ---- tanh -> fp16; sumsq on ACT; rinv; fp16 scales ----
            ah = mp.tile([P, T, D], HT)
            bh = [mp.tile([P, T, D], HT, name=f"bh{h}") for h in range(2)]
            ssq = mp.tile([P, 3, T], DT)
            nrm = mp.tile([P, 3, T], DT)
            rinv = mp.tile([P, 3, T], DT)
            blocks = [(a_sb, ah, 0), (b_sb[0], bh[0], 1), (b_sb[1], bh[1], 2)]
            aT = mp.tile([P, T * KH, P], HT)       # [d, i*2+kh, a]
            bT = mp.tile([P, T, KH, 2, P], HT)     # [d, j, kh, half, b]
            # per-block pipeline: tanh (ACT) -> sumsq (DVE) -> sqrt (one ACT
            # instr per block) -> recip -> scales -> XBAR transposes, so each
            # block's transposed operands are ready as soon as its load is.
            for x_sb, xh, bi in blocks:
                for q in range(4):
                    sl = slice(q * 4, (q + 1) * 4)
                    nc.scalar.activation(xh[:, sl, :], x_sb[:, sl, :],
                                         ACTF.Tanh)
                    sq = wp.tile([P, 4, D], HT, name=f"sq{bi}_{q}", tag="sq")
                    nc.vector.tensor_tensor(sq[:, :, :], xh[:, sl, :],
                                            xh[:, sl, :], ALU.mult)
                    nc.vector.tensor_reduce(ssq[:, bi, sl], sq[:, :, :],
                                            mybir.AxisListType.X, ALU.add)
                nc.scalar.activation(nrm[:, bi, :], ssq[:, bi, :], ACTF.Sqrt)
                nc.vector.reciprocal(rinv[:, bi, :], nrm[:, bi, :])
                for q in range(4):
                    for i in range(q * 4, (q + 1) * 4):
                        nc.vector.tensor_scalar_mul(xh[:, i, :], xh[:, i, :],
                                                    rinv[:, bi, i:i + 1])
                    sl = slice(q * 4, (q + 1) * 4)
                    if bi == 0:
                        nc.sync.dma_start_transpose(
                            aT[:, q * 8:(q + 1) * 8, :], xh[:, sl, :])
                    else:
                        nc.sync.dma_start_transpose(
                            bT[:, sl, :, bi - 1, :], xh[:, sl, :])

            def amat(i0, kh):
                return aT[:, i0 * KH + kh, :]

            def bmov(j0, w, kh):
                return bT[:, j0:j0 + w, kh, :, :]

            # ---- DP state ----
            # Step k: S = 17-k slots per pair: [0 inject, c=1..16-k data
            # (cell j = k+c)]; regions packed [row | col], pair-major.
            RC0 = mp.tile([P, 2 * 16 * F], DT, tag="ld_a")
            RC1 = mp.tile([P, 2 * 16 * F], DT, tag="ld_b0")
            LRC = mp.tile([P, 2 * 16 * F], DT, tag="ld_b1")
            ud = mp.tile([P, F], DT)

            def reg(tile, k, w):
                S = 17 - k
                return tile[:, w * S * F:(w + 1) * S * F].rearrange(
                    "p (f s) -> p s f", s=S)

            def chain2d(tile, k, w):
                S = 17 - k
                return tile[:, w * S * F:(w + 1) * S * F]

            def both(tile, k):
                S = 17 - k
                return tile[:, 0:2 * S * F].rearrange(
                    "p (w f s) -> p w f s", w=2, s=S)

            rc_prev = None
            m1 = [wp.tile([P, F], DT, name=f"m1_{x}", tag="m1")
                  for x in range(2)]

            with tc.tile_pool(name="pp", bufs=1, space="PSUM") as pp:
                for k in range(1, T + 1):
                    S = 17 - k
                    nd = 16 - k          # data slots per region

                    # --- diag plane (a-frame k-1 x b-frame k-1) ---
                    psd = pp.tile([128, F], DT, name=f"psd{k}", tag="locd",
                                  bufs=2)
                    for kh in range(KH):
                        nc.tensor.matmul(psd[:, :], amat(k - 1, kh),
                                         bmov(k - 1, 1, kh),
                                         start=(kh == 0), stop=(kh == KH - 1))

                    rc_cur = RC0 if (k % 2) else RC1
                    if k == 1:
                        nc.gpsimd.memset(both(rc_cur, k)[:, :, :, 0:1], BIAS)
                        nc.gpsimd.memset(both(rc_cur, k)[:, :, :, 1:S], 0.0)
                    else:
                        mm = m1[k % 2]
                        nc.vector.tensor_tensor(
                            mm[:, :], reg(rc_prev, k - 1, 0)[:, 1, :],
                            reg(rc_prev, k - 1, 1)[:, 1, :], ALU.max)
                        nc.vector.scalar_tensor_tensor(
                            mm[:, :], mm[:, :], float((k - 1) / k),
                            reg(rc_prev, k - 1, 0)[:, 0, :],
                            ALU.mult, ALU.max)
                        if k == T:
                            # u[16,16] = max(m1*15/16, udp) + lc_diag
                            nc.vector.tensor_tensor(ud[:, :], mm[:, :],
                                                    psd[:, :], ALU.add)
                            break
                        # inject d0 = mm + 64 into slot 0 of both regions
                        # (on DVE: keeps ACT out of the per-step loop)
                        nc.vector.tensor_scalar_add(
                            both(rc_cur, k)[:, :, :, 0],
                            mm[:, :].unsqueeze(1).broadcast_to([P, 2, F]),
                            BIAS)

                    # --- diag eviction: inject d1 = lc_diag - 64 ---
                    for w in range(2):
                        nc.scalar.activation(reg(LRC, k, w)[:, 0, :],
                                             psd[:, :], ACTF.Copy, bias=-BIAS)

                    # --- border matmuls + evictions ---
                    # row data planes: b-frames k-1+c (c=1..nd) -> slots c
                    for c0 in range(1, nd + 1, 2):
                        w = min(2, nd + 1 - c0)
                        psv = pp.tile([128, 2 * F], DT, name=f"psr{k}_{c0}",
                                      tag="locr", bufs=3)
                        for kh in range(KH):
                            nc.tensor.matmul(
                                psv[:, 0:w * F], amat(k - 1, kh),
                                bmov(k - 1 + c0, w, kh),
                                start=(kh == 0), stop=(kh == KH - 1))
                        nc.scalar.activation(
                            reg(LRC, k, 0)[:, c0:c0 + w, :],
                            psv[:, 0:w * F].rearrange("p (n f) -> p n f", f=F),
                            ACTF.Copy)
                    # col data planes: a-frames k-1+c (c=1..nd)
                    for c0 in range(1, nd + 1, 2):
                        w = min(2, nd + 1 - c0)
                        psv = pp.tile([128, 2 * F], DT, name=f"psc{k}_{c0}",
                                      tag="locc", bufs=3)
                        for q in range(w):
                            for kh in range(KH):
                                nc.tensor.matmul(
                                    psv[:, q * F:(q + 1) * F],
                                    amat(k - 1 + c0 + q, kh),
                                    bmov(k - 1, 1, kh),
                                    start=(kh == 0), stop=(kh == KH - 1))
                        nc.scalar.activation(
                            reg(LRC, k, 1)[:, c0:c0 + w, :],
                            psv[:, 0:w * F].rearrange("p (n f) -> p n f", f=F),
                            ACTF.Copy)

                    # --- preps: t[c] = max(prev[c+1]*(j-1)/j, prev[c]) ---
                    if k > 1:
                        bc = both(rc_cur, k)
                        bp = both(rc_prev, k - 1)
                        for c in range(1, nd + 1):
                            j = k + c
                            nc.vector.scalar_tensor_tensor(
                                bc[:, :, :, c], bp[:, :, :, c + 1],
                                float((j - 1) / j), bp[:, :, :, c],
                                ALU.mult, ALU.max)

                    # --- scans (separate per region so the row scan can
                    # start while col evictions still stream) ---
                    for w in range(2):
                        nc.vector.tensor_tensor_scan(
                            chain2d(rc_cur, k, w), chain2d(rc_cur, k, w),
                            chain2d(LRC, k, w), 0.0, ALU.max, ALU.add)

                    rc_prev = rc_cur

            out_sb = mp.tile([P, F], DT)
            nc.vector.tensor_scalar_mul(out_sb[:, :], ud[:, :], 1.0 / T)
            nc.sync.dma_start(out_d.ap(), out_sb[:, :])

    nc.compile()
    return nc


def kernel(a: np.ndarray, b: np.ndarray) -> np.ndarray:
    a = np.ascontiguousarray(a, dtype=np.float32)
    b = np.ascontiguousarray(b, dtype=np.float32)
    assert a.shape == (NA, T, D) and b.shape == (NB, T, D)

    nc = build_program()

    in_maps = []
    for core in range(8):
        ca, cb = core // 2, core % 2
        in_maps.append({
            "a_c": a[ca * ACH:(ca + 1) * ACH],
            "b_c": b[cb * BCH:(cb + 1) * BCH],
        })

    res = bass_utils.run_bass_kernel_spmd(nc, in_maps, core_ids=list(range(8)))
    global _last_results
    _last_results = res

    out = np.zeros((NA, NB), dtype=np.float32)
    for core in range(8):
        ca, cb = core // 2, core % 2
        out[ca * ACH:(ca + 1) * ACH, cb * BCH:(cb + 1) * BCH] = \
            res.results[core]["out"]
    return out


# revision 12
# speedup vs baseline: 1.0392x; 1.0166x over previous
"""Trainium2 Bass kernel for nn_DynamicMaxSimilarity — v4.

Full inputs a,b: [512, 16, 256] f32.
  an = l2norm(tanh(a)) rows; bn likewise
  sim[a,b,i,j] = dot(an[a,i], bn[b,j]);  out[a,b] = DTW-like max-avg DP.

Sharding: 8 cores as 4 a-chunks (128) x 2 b-chunks (256). Per-core block
[128 a, 256 b]; pairs live as [128 partitions (a), 256 free (b)].

v4 vs baseline:
- fp16 normalize/matmul path (tanh casts to fp16; matmuls run fp16).
- XBAR DMA transposes (dma_start_transpose) replace PE transposes and
  their PSUM->SBUF evictions; DMA engines are otherwise idle.
- ACT table plan: all Tanh first, one switch to the Sqrt set, then only
  Copy/Square (in every set) -> 2 table loads instead of 14.
- sumsq via ACT Square+accum_out per frame (off the DVE critical path);
  row scales via fp16 tensor_scalar (4x DVE mode).
- DP unchanged math (scaled domain u = si*max(i,j), fp32 state) but
  single-slot chain injects: per pair chain slots [inject, data...]:
    inject: d0 = max(udp, m1*(k-1)/k) + 64 (ACT write, bias),
            d1 = lc_diag - 64 (diag eviction with bias=-64)
    scan:   y = max(d0, stale) + d1 = u[k,k]   (|u| <= 16 < 48 <= d0)
  which drops one slot per chain per step vs the baseline's two-slot
  scheme and removes the -BIG memsets and D2 copies.
"""

import numpy as np

import concourse.bass as bass
from concourse import bacc
import concourse.mybir as mybir
from concourse.tile import TileContext
from concourse import bass_utils

NA, NB, T, D = 512, 512, 16, 256
ACH, BCH = 128, 256
P = 128
F = BCH              # pairs per partition
KH = D // 128
DT = mybir.dt.float32
HT = mybir.dt.float16
ALU = mybir.AluOpType
ACTF = mybir.ActivationFunctionType
BIAS = 64.0

_last_results = None


def build_program():
    nc = bacc.Bacc("TRN2", target_bir_lowering=False, debug=False)

    a_d = nc.dram_tensor("a_c", [ACH, T, D], DT, kind="ExternalInput")
    b_d = nc.dram_tensor("b_c", [BCH, T, D], DT, kind="ExternalInput")
    out_d = nc.dram_tensor("out", [ACH, BCH], DT, kind="ExternalOutput")

    with TileContext(nc) as tc:
        with (
            tc.tile_pool(name="mp", bufs=1) as mp,
            tc.tile_pool(name="wp", bufs=2) as wp,
        ):
            # ---- load (fp32), quarters interleaved across blocks so every
            # block's normalize pipeline starts as early as possible ----
            a_sb = mp.tile([P, T, D], DT, tag="ld_a")
            b_sb = [mp.tile([P, T, D], DT, name=f"b_sb{h}", tag=f"ld_b{h}")
                    for h in range(2)]
            for q in range(4):
                nc.sync.dma_start(a_sb[:, q * 4:(q + 1) * 4, :],
                                  a_d.ap()[:, q * 4:(q + 1) * 4, :])
                for h in range(2):
                    nc.sync.dma_start(
                        b_sb[h][:, q * 4:(q + 1) * 4, :],
                        b_d.ap()[h * 128:(h + 1) * 128, q * 4:(q + 1) * 4, :])

            # ---- tanh -> fp16; sumsq on ACT; rinv; fp16 scales ----
            ah = mp.tile([P, T, D], HT)
            bh = [mp.tile([P, T, D], HT, name=f"bh{h}") for h in range(2)]
            ssq = mp.tile([P, 3, T], DT)
            nrm = mp.tile([P, 3, T], DT)
            rinv = mp.tile([P, 3, T], DT)
            blocks = [(a_sb, ah, 0), (b_sb[0], bh[0], 1), (b_sb[1], bh[1], 2)]
            aT = mp.tile([P, T * KH, P], HT)       # [d, i*2+kh, a]
            bT = mp.tile([P, T, KH, 2, P], HT)     # [d, j, kh, half, b]
            # per-block pipeline: tanh (ACT) -> sumsq (DVE) -> sqrt (one ACT
            # instr per block) -> recip -> scales -> XBAR transposes, so each
            # block's transposed operands are ready as soon as its load is.
            for x_sb, xh, bi in blocks:
                for q in range(4):
                    sl = slice(q * 4, (q + 1) * 4)
                    nc.scalar.activation(xh[:, sl, :], x_sb[:, sl, :],
                                         ACTF.Tanh)
                    # sumsq split: frame 4q on ACT (Square+accum, same
                    # table set), frames 4q+1..3 on DVE -- both fit the
                    # load cadence so neither serializes the head
                    sqa = wp.tile([P, D], HT, name=f"sqa{bi}_{q}", tag="sqa")
                    nc.scalar.activation(sqa[:, :], xh[:, q * 4, :],
                                         ACTF.Square,
                                         accum_out=ssq[:, bi, q * 4:q * 4 + 1])
                    sq = wp.tile([P, 3, D], HT, name=f"sq{bi}_{q}", tag="sq")
                    sl3 = slice(q * 4 + 1, (q + 1) * 4)
                    nc.vector.tensor_tensor(sq[:, :, :], xh[:, sl3, :],
                                            xh[:, sl3, :], ALU.mult)
                    nc.vector.tensor_reduce(ssq[:, bi, sl3], sq[:, :, :],
                                            mybir.AxisListType.X, ALU.add)
                nc.scalar.activation(nrm[:, bi, :], ssq[:, bi, :], ACTF.Sqrt)
                nc.vector.reciprocal(rinv[:, bi, :], nrm[:, bi, :])
                for q in range(4):
                    for i in range(q * 4, (q + 1) * 4):
                        nc.vector.tensor_scalar_mul(xh[:, i, :], xh[:, i, :],
                                                    rinv[:, bi, i:i + 1])
                    sl = slice(q * 4, (q + 1) * 4)
                    if bi == 0:
                        nc.sync.dma_start_transpose(
                            aT[:, q * 8:(q + 1) * 8, :], xh[:, sl, :])
                    else:
                        nc.sync.dma_start_transpose(
                            bT[:, sl, :, bi - 1, :], xh[:, sl, :])

            def amat(i0, kh):
                return aT[:, i0 * KH + kh, :]

            def bmov(j0, w, kh):
                return bT[:, j0:j0 + w, kh, :, :]

            # ---- DP state ----
            # Step k: S = 17-k slots per pair: [0 inject, c=1..16-k data
            # (cell j = k+c)]; regions packed [row | col], pair-major.
            RC0 = mp.tile([P, 2 * 16 * F], DT, tag="ld_a")
            RC1 = mp.tile([P, 2 * 16 * F], DT, tag="ld_b0")
            LRC = mp.tile([P, 2 * 16 * F], DT, tag="ld_b1")
            ud = mp.tile([P, F], DT)

            def reg(tile, k, w):
                S = 17 - k
                return tile[:, w * S * F:(w + 1) * S * F].rearrange(
                    "p (f s) -> p s f", s=S)

            def chain2d(tile, k, w):
                S = 17 - k
                return tile[:, w * S * F:(w + 1) * S * F]

            def both(tile, k):
                S = 17 - k
                return tile[:, 0:2 * S * F].rearrange(
                    "p (w f s) -> p w f s", w=2, s=S)

            rc_prev = None
            m1 = [wp.tile([P, F], DT, name=f"m1_{x}", tag="m1")
                  for x in range(2)]

            with tc.tile_pool(name="pp", bufs=1, space="PSUM") as pp:
                for k in range(1, T + 1):
                    S = 17 - k
                    nd = 16 - k          # data slots per region

                    # --- diag plane (a-frame k-1 x b-frame k-1) ---
                    psd = pp.tile([128, F], DT, name=f"psd{k}", tag="locd",
                                  bufs=2)
                    for kh in range(KH):
                        nc.tensor.matmul(psd[:, :], amat(k - 1, kh),
                                         bmov(k - 1, 1, kh),
                                         start=(kh == 0), stop=(kh == KH - 1))

                    rc_cur = RC0 if (k % 2) else RC1
                    if k == 1:
                        nc.gpsimd.memset(both(rc_cur, k)[:, :, :, 0:1], BIAS)
                        nc.gpsimd.memset(both(rc_cur, k)[:, :, :, 1:S], 0.0)
                    else:
                        mm = m1[k % 2]
                        nc.vector.tensor_tensor(
                            mm[:, :], reg(rc_prev, k - 1, 0)[:, 1, :],
                            reg(rc_prev, k - 1, 1)[:, 1, :], ALU.max)
                        nc.vector.scalar_tensor_tensor(
                            mm[:, :], mm[:, :], float((k - 1) / k),
                            reg(rc_prev, k - 1, 0)[:, 0, :],
                            ALU.mult, ALU.max)
                        if k == T:
                            # u[16,16] = max(m1*15/16, udp) + lc_diag
                            nc.vector.tensor_tensor(ud[:, :], mm[:, :],
                                                    psd[:, :], ALU.add)
                            break
                        # inject d0 = mm + 64 into slot 0 of both regions
                        # (on DVE: keeps ACT out of the per-step loop)
                        nc.vector.tensor_scalar_add(
                            both(rc_cur, k)[:, :, :, 0],
                            mm[:, :].unsqueeze(1).broadcast_to([P, 2, F]),
                            BIAS)

                    # --- diag eviction: inject d1 = lc_diag - 64 ---
                    for w in range(2):
                        nc.scalar.activation(reg(LRC, k, w)[:, 0, :],
                                             psd[:, :], ACTF.Copy, bias=-BIAS)

                    # --- border matmuls + evictions ---
                    # row data planes: b-frames k-1+c (c=1..nd) -> slots c
                    for c0 in range(1, nd + 1, 2):
                        w = min(2, nd + 1 - c0)
                        psv = pp.tile([128, 2 * F], DT, name=f"psr{k}_{c0}",
                                      tag="locr", bufs=3)
                        for kh in range(KH):
                            nc.tensor.matmul(
                                psv[:, 0:w * F], amat(k - 1, kh),
                                bmov(k - 1 + c0, w, kh),
                                start=(kh == 0), stop=(kh == KH - 1))
                        nc.scalar.activation(
                            reg(LRC, k, 0)[:, c0:c0 + w, :],
                            psv[:, 0:w * F].rearrange("p (n f) -> p n f", f=F),
                            ACTF.Copy)
                    # col data planes: a-frames k-1+c (c=1..nd)
                    for c0 in range(1, nd + 1, 2):
                        w = min(2, nd + 1 - c0)
                        psv = pp.tile([128, 2 * F], DT, name=f"psc{k}_{c0}",
                                      tag="locc", bufs=3)
                        for q in range(w):
                            for kh in range(KH):
                                nc.tensor.matmul(
                                    psv[:, q * F:(q + 1) * F],
                                    amat(k - 1 + c0 + q, kh),
                                    bmov(k - 1, 1, kh),
                                    start=(kh == 0), stop=(kh == KH - 1))
                        nc.scalar.activation(
                            reg(LRC, k, 1)[:, c0:c0 + w, :],
                            psv[:, 0:w * F].rearrange("p (n f) -> p n f", f=F),
                            ACTF.Copy)

                    # --- preps: t[c] = max(prev[c+1]*(j-1)/j, prev[c]) ---
                    if k > 1:
                        bc = both(rc_cur, k)
                        bp = both(rc_prev, k - 1)
                        for c in range(1, nd + 1):
                            j = k + c
                            nc.vector.scalar_tensor_tensor(
                                bc[:, :, :, c], bp[:, :, :, c + 1],
                                float((j - 1) / j), bp[:, :, :, c],
                                ALU.mult, ALU.max)

                    # --- scans (separate per region so the row scan can
                    # start while col evictions still stream) ---
                    for w in range(2):
                        nc.vector.tensor_tensor_scan(
                            chain2d(rc_cur, k, w), chain2d(rc_cur, k, w),
                            chain2d(LRC, k, w), 0.0, ALU.max, ALU.add)

                    rc_prev = rc_cur

            out_sb = mp.tile([P, F], DT)
            nc.vector.tensor_scalar_mul(out_sb[:, :], ud[:, :], 1.0 / T)
            nc.sync.dma_start(out_d.ap(), out_sb[:, :])

    nc.compile()
    return nc


def kernel(a: np.ndarray, b: np.ndarray) -> np.ndarray:
    a = np.ascontiguousarray(a, dtype=np.float32)
    b = np.ascontiguousarray(b, dtype=np.float32)
    assert a.shape == (NA, T, D) and b.shape == (NB, T, D)

    nc = build_program()

    in_maps = []
    for core in range(8):
        ca, cb = core // 2, core % 2
        in_maps.append({
            "a_c": a[ca * ACH:(ca + 1) * ACH],
            "b_c": b[cb * BCH:(cb + 1) * BCH],
        })

    res = bass_utils.run_bass_kernel_spmd(nc, in_maps, core_ids=list(range(8)))
    global _last_results
    _last_results = res

    out = np.zeros((NA, NB), dtype=np.float32)
    for core in range(8):
        ca, cb = core // 2, core % 2
        out[ca * ACH:(ca + 1) * ACH, cb * BCH:(cb + 1) * BCH] = \
            res.results[core]["out"]
    return out


# revision 14
# speedup vs baseline: 1.0650x; 1.0248x over previous
"""Trainium2 Bass kernel for nn_DynamicMaxSimilarity — v4.

Full inputs a,b: [512, 16, 256] f32.
  an = l2norm(tanh(a)) rows; bn likewise
  sim[a,b,i,j] = dot(an[a,i], bn[b,j]);  out[a,b] = DTW-like max-avg DP.

Sharding: 8 cores as 4 a-chunks (128) x 2 b-chunks (256). Per-core block
[128 a, 256 b]; pairs live as [128 partitions (a), 256 free (b)].

v4 vs baseline:
- fp16 normalize/matmul path (tanh casts to fp16; matmuls run fp16).
- XBAR DMA transposes (dma_start_transpose) replace PE transposes and
  their PSUM->SBUF evictions; DMA engines are otherwise idle.
- ACT table plan: all Tanh first, one switch to the Sqrt set, then only
  Copy/Square (in every set) -> 2 table loads instead of 14.
- sumsq via ACT Square+accum_out per frame (off the DVE critical path);
  row scales via fp16 tensor_scalar (4x DVE mode).
- DP unchanged math (scaled domain u = si*max(i,j), fp32 state) but
  single-slot chain injects: per pair chain slots [inject, data...]:
    inject: d0 = max(udp, m1*(k-1)/k) + 64 (ACT write, bias),
            d1 = lc_diag - 64 (diag eviction with bias=-64)
    scan:   y = max(d0, stale) + d1 = u[k,k]   (|u| <= 16 < 48 <= d0)
  which drops one slot per chain per step vs the baseline's two-slot
  scheme and removes the -BIG memsets and D2 copies.
"""

import numpy as np

import concourse.bass as bass
from concourse import bacc
import concourse.mybir as mybir
from concourse.tile import TileContext
from concourse import bass_utils

NA, NB, T, D = 512, 512, 16, 256
ACH, BCH = 128, 256
P = 128
F = BCH              # pairs per partition
KH = D // 128
DT = mybir.dt.float32
HT = mybir.dt.float16
ALU = mybir.AluOpType
ACTF = mybir.ActivationFunctionType
BIAS = 64.0

_last_results = None


def build_program():
    nc = bacc.Bacc("TRN2", target_bir_lowering=False, debug=False)

    a_d = nc.dram_tensor("a_c", [ACH, T, D], DT, kind="ExternalInput")
    b_d = nc.dram_tensor("b_c", [BCH, T, D], DT, kind="ExternalInput")
    out_d = nc.dram_tensor("out", [ACH, BCH], DT, kind="ExternalOutput")

    with TileContext(nc) as tc:
        with (
            tc.tile_pool(name="mp", bufs=1) as mp,
            tc.tile_pool(name="wp", bufs=2) as wp,
        ):
            # ---- load (fp32), quarters interleaved across blocks so every
            # block's normalize pipeline starts as early as possible ----
            a_sb = mp.tile([P, T, D], DT, tag="ld_a")
            b_sb = [mp.tile([P, T, D], DT, name=f"b_sb{h}", tag=f"ld_b{h}")
                    for h in range(2)]
            for q in range(4):
                nc.sync.dma_start(a_sb[:, q * 4:(q + 1) * 4, :],
                                  a_d.ap()[:, q * 4:(q + 1) * 4, :])
            for h in range(2):
                for q in range(4):
                    nc.sync.dma_start(
                        b_sb[h][:, q * 4:(q + 1) * 4, :],
                        b_d.ap()[h * 128:(h + 1) * 128, q * 4:(q + 1) * 4, :])

            # ---- tanh -> fp16; sumsq on ACT; rinv; fp16 scales ----
            ah = mp.tile([P, T, D], HT)
            bh = [mp.tile([P, T, D], HT, name=f"bh{h}") for h in range(2)]
            ssq = mp.tile([P, 3, T], DT)
            nrm = mp.tile([P, 3, T], DT)
            rinv = mp.tile([P, 3, T], DT)
            blocks = [(a_sb, ah, 0), (b_sb[0], bh[0], 1), (b_sb[1], bh[1], 2)]
            aT = mp.tile([P, T * KH, P], HT)       # [d, i*2+kh, a]
            bT = mp.tile([P, T, KH, 2, P], HT)     # [d, j, kh, half, b]
            # per-block pipeline: tanh (ACT) -> sumsq (DVE) -> sqrt (one ACT
            # instr per block) -> recip -> scales -> XBAR transposes, so each
            # block's transposed operands are ready as soon as its load is.
            for x_sb, xh, bi in blocks:
                for q in range(4):
                    sl = slice(q * 4, (q + 1) * 4)
                    nc.scalar.activation(xh[:, sl, :], x_sb[:, sl, :],
                                         ACTF.Tanh)
                    # sumsq split: frame 4q on ACT (Square+accum, same
                    # table set), frames 4q+1..3 on DVE -- both fit the
                    # load cadence so neither serializes the head
                    sqa = wp.tile([P, D], HT, name=f"sqa{bi}_{q}", tag="sqa")
                    nc.scalar.activation(sqa[:, :], xh[:, q * 4, :],
                                         ACTF.Square,
                                         accum_out=ssq[:, bi, q * 4:q * 4 + 1])
                    sq = wp.tile([P, 3, D], HT, name=f"sq{bi}_{q}", tag="sq")
                    sl3 = slice(q * 4 + 1, (q + 1) * 4)
                    nc.vector.tensor_tensor(sq[:, :, :], xh[:, sl3, :],
                                            xh[:, sl3, :], ALU.mult)
                    nc.vector.tensor_reduce(ssq[:, bi, sl3], sq[:, :, :],
                                            mybir.AxisListType.X, ALU.add)
                # rinv = rsqrt(ssq) via the int bit trick + 2 Newton steps
                # (all-DVE: avoids the Sqrt ACT table load entirely)
                sv = ssq[:, bi, :]
                yv = rinv[:, bi, :]
                wv = nrm[:, bi, :]
                nc.vector.tensor_scalar(yv.bitcast(mybir.dt.int32),
                                        sv.bitcast(mybir.dt.int32),
                                        1, None, ALU.logical_shift_right)
                nc.vector.tensor_scalar(yv.bitcast(mybir.dt.int32),
                                        yv.bitcast(mybir.dt.int32),
                                        0x5F3759DF, -1, ALU.subtract, ALU.mult)
                for _ in range(2):
                    nc.vector.tensor_tensor(wv, yv, yv, ALU.mult)
                    nc.vector.tensor_tensor(wv, wv, sv, ALU.mult)
                    nc.vector.tensor_scalar(wv, wv, -0.5, 1.5,
                                            ALU.mult, ALU.add)
                    nc.vector.tensor_tensor(yv, yv, wv, ALU.mult)
                for q in range(4):
                    for i in range(q * 4, (q + 1) * 4):
                        nc.vector.tensor_scalar_mul(xh[:, i, :], xh[:, i, :],
                                                    rinv[:, bi, i:i + 1])
                    sl = slice(q * 4, (q + 1) * 4)
                    if bi == 0:
                        nc.sync.dma_start_transpose(
                            aT[:, q * 8:(q + 1) * 8, :], xh[:, sl, :])
                    else:
                        nc.sync.dma_start_transpose(
                            bT[:, sl, :, bi - 1, :], xh[:, sl, :])

            def amat(i0, kh):
                return aT[:, i0 * KH + kh, :]

            def bmov(j0, w, kh):
                return bT[:, j0:j0 + w, kh, :, :]

            # ---- DP state ----
            # Step k: S = 17-k slots per pair: [0 inject, c=1..16-k data
            # (cell j = k+c)]; regions packed [row | col], pair-major.
            RC0 = mp.tile([P, 2 * 16 * F], DT, tag="ld_a")
            RC1 = mp.tile([P, 2 * 16 * F], DT, tag="ld_b0")
            LRC = mp.tile([P, 2 * 16 * F], DT, tag="ld_b1")
            ud = mp.tile([P, F], DT)

            def reg(tile, k, w):
                S = 17 - k
                return tile[:, w * S * F:(w + 1) * S * F].rearrange(
                    "p (f s) -> p s f", s=S)

            def chain2d(tile, k, w):
                S = 17 - k
                return tile[:, w * S * F:(w + 1) * S * F]

            def both(tile, k):
                S = 17 - k
                return tile[:, 0:2 * S * F].rearrange(
                    "p (w f s) -> p w f s", w=2, s=S)

            rc_prev = None
            m1 = [wp.tile([P, F], DT, name=f"m1_{x}", tag="m1")
                  for x in range(2)]

            with tc.tile_pool(name="pp", bufs=1, space="PSUM") as pp:
                for k in range(1, T + 1):
                    S = 17 - k
                    nd = 16 - k          # data slots per region

                    # --- diag plane (a-frame k-1 x b-frame k-1) ---
                    psd = pp.tile([128, F], DT, name=f"psd{k}", tag="locd",
                                  bufs=2)
                    for kh in range(KH):
                        nc.tensor.matmul(psd[:, :], amat(k - 1, kh),
                                         bmov(k - 1, 1, kh),
                                         start=(kh == 0), stop=(kh == KH - 1))

                    rc_cur = RC0 if (k % 2) else RC1
                    if k == 1:
                        nc.gpsimd.memset(both(rc_cur, k)[:, :, :, 0:1], BIAS)
                        nc.gpsimd.memset(both(rc_cur, k)[:, :, :, 1:S], 0.0)
                    else:
                        mm = m1[k % 2]
                        nc.vector.tensor_tensor(
                            mm[:, :], reg(rc_prev, k - 1, 0)[:, 1, :],
                            reg(rc_prev, k - 1, 1)[:, 1, :], ALU.max)
                        nc.vector.scalar_tensor_tensor(
                            mm[:, :], mm[:, :], float((k - 1) / k),
                            reg(rc_prev, k - 1, 0)[:, 0, :],
                            ALU.mult, ALU.max)
                        if k == T:
                            # u[16,16] = max(m1*15/16, udp) + lc_diag
                            nc.vector.tensor_tensor(ud[:, :], mm[:, :],
                                                    psd[:, :], ALU.add)
                            break
                        # inject d0 = mm + 64 into slot 0 of both regions
                        # (on DVE: keeps ACT out of the per-step loop)
                        nc.vector.tensor_scalar_add(
                            both(rc_cur, k)[:, :, :, 0],
                            mm[:, :].unsqueeze(1).broadcast_to([P, 2, F]),
                            BIAS)

                    # --- diag eviction: inject d1 = lc_diag - 64 ---
                    for w in range(2):
                        nc.scalar.activation(reg(LRC, k, w)[:, 0, :],
                                             psd[:, :], ACTF.Copy, bias=-BIAS)

                    # --- border matmuls + evictions ---
                    # row data planes: b-frames k-1+c (c=1..nd) -> slots c
                    for c0 in range(1, nd + 1, 2):
                        w = min(2, nd + 1 - c0)
                        psv = pp.tile([128, 2 * F], DT, name=f"psr{k}_{c0}",
                                      tag="locr", bufs=3)
                        for kh in range(KH):
                            nc.tensor.matmul(
                                psv[:, 0:w * F], amat(k - 1, kh),
                                bmov(k - 1 + c0, w, kh),
                                start=(kh == 0), stop=(kh == KH - 1))
                        nc.scalar.activation(
                            reg(LRC, k, 0)[:, c0:c0 + w, :],
                            psv[:, 0:w * F].rearrange("p (n f) -> p n f", f=F),
                            ACTF.Copy)
                    # col data planes: a-frames k-1+c (c=1..nd)
                    for c0 in range(1, nd + 1, 2):
                        w = min(2, nd + 1 - c0)
                        psv = pp.tile([128, 2 * F], DT, name=f"psc{k}_{c0}",
                                      tag="locc", bufs=3)
                        for q in range(w):
                            for kh in range(KH):
                                nc.tensor.matmul(
                                    psv[:, q * F:(q + 1) * F],
                                    amat(k - 1 + c0 + q, kh),
                                    bmov(k - 1, 1, kh),
                                    start=(kh == 0), stop=(kh == KH - 1))
                        nc.scalar.activation(
                            reg(LRC, k, 1)[:, c0:c0 + w, :],
                            psv[:, 0:w * F].rearrange("p (n f) -> p n f", f=F),
                            ACTF.Copy)

                    # --- preps: t[c] = max(prev[c+1]*(j-1)/j, prev[c]) ---
                    if k > 1:
                        bc = both(rc_cur, k)
                        bp = both(rc_prev, k - 1)
                        for c in range(1, nd + 1):
                            j = k + c
                            nc.vector.scalar_tensor_tensor(
                                bc[:, :, :, c], bp[:, :, :, c + 1],
                                float((j - 1) / j), bp[:, :, :, c],
                                ALU.mult, ALU.max)

                    # --- scans (separate per region so the row scan can
                    # start while col evictions still stream) ---
                    for w in range(2):
                        nc.vector.tensor_tensor_scan(
                            chain2d(rc_cur, k, w), chain2d(rc_cur, k, w),
                            chain2d(LRC, k, w), 0.0, ALU.max, ALU.add)

                    rc_prev = rc_cur

            out_sb = mp.tile([P, F], DT)
            nc.vector.tensor_scalar_mul(out_sb[:, :], ud[:, :], 1.0 / T)
            nc.sync.dma_start(out_d.ap(), out_sb[:, :])

    nc.compile()
    return nc


def kernel(a: np.ndarray, b: np.ndarray) -> np.ndarray:
    a = np.ascontiguousarray(a, dtype=np.float32)
    b = np.ascontiguousarray(b, dtype=np.float32)
    assert a.shape == (NA, T, D) and b.shape == (NB, T, D)

    nc = build_program()

    in_maps = []
    for core in range(8):
        ca, cb = core // 2, core % 2
        in_maps.append({
            "a_c": a[ca * ACH:(ca + 1) * ACH],
            "b_c": b[cb * BCH:(cb + 1) * BCH],
        })

    res = bass_utils.run_bass_kernel_spmd(nc, in_maps, core_ids=list(range(8)))
    global _last_results
    _last_results = res

    out = np.zeros((NA, NB), dtype=np.float32)
    for core in range(8):
        ca, cb = core // 2, core % 2
        out[ca * ACH:(ca + 1) * ACH, cb * BCH:(cb + 1) * BCH] = \
            res.results[core]["out"]
    return out
